# revision 10
# baseline (speedup 1.0000x reference)
"""Multi-head causal attention (B=2, S=2048, H=16, Dh=64) on 8 TRN2 NeuronCores.

Sharding: (batch, head-quad) — core c owns batch c//4 and heads
4*(c%4)..4*(c%4)+3 (a 256-wide feature block). Each core computes its
heads' QKV projections over its batch's 2048 tokens, causal attention,
and a partial output projection (attn_c @ Wo[:, mslice].T); the host
sums the 4 partials per batch. vs the previous 2-head x 2-batch TP
sharding this halves both the x reads (8MB not 16MB) and the fp32
partial-out writes (8MB not 16MB) per core -- DMA was the measured
bottleneck (~150GB/s effective on 1-2KB-line transfers).

On-chip: feature dim on SBUF partitions throughout. Scores are computed
as S.T[j, i] per 128-j-tile x 512-i-chunk; the two heads of a pair live
on partitions 0-63 / 64-127 so their score matmuls go to disjoint PE
row-groups and run concurrently. exp is one merged [128,1024] ACT
instruction per (jt, head-pair) (amortizes the 352-cycle ACT fixed
cost). p and v are bf16 (full-rate PE, half-cost DVE mask); softmax
row-sum comes free from 64 ones-columns appended to V in the PV matmul.
"""

import numpy as np

import concourse.bass as bass
import concourse.mybir as mybir
import concourse.tile as tile
from concourse import bacc
from concourse.bass import ds
from concourse.masks import make_identity

B, S, H, Dh = 2, 2048, 16, 64
D = H * Dh            # 1024
NCORES = 8
HPC = 4               # heads per core
M = HPC * Dh          # per-core feature block = 256
IC = 512              # i-chunk (matmul moving free dim)
NICB = S // IC        # 4 i-chunks
NJT = S // 128        # 16 j-tiles

F32 = mybir.dt.float32
F32R = mybir.dt.float32r
BF16 = mybir.dt.bfloat16

XDT = BF16            # x activations + QKV weights (DMA + full-rate PE)
PDT = BF16            # exp(p) and v in the PV matmul
ODT = F32             # output partials must stay f32 (2-byte DMA writes slow)
AF = mybir.ActivationFunctionType
ALU = mybir.AluOpType


def _build_bass(bench_iters=None):
    nc = bacc.Bacc("TRN2", target_bir_lowering=False, debug=False,
                   num_devices=NCORES)

    xqT = nc.dram_tensor("xqT", [D, S], XDT, kind="ExternalInput").ap()
    xkT = nc.dram_tensor("xkT", [D, S], XDT, kind="ExternalInput").ap()
    wqT = nc.dram_tensor("wqT", [D, M], XDT, kind="ExternalInput").ap()
    wkT = nc.dram_tensor("wkT", [D, M], XDT, kind="ExternalInput").ap()
    wvT = nc.dram_tensor("wvT", [D, M], XDT, kind="ExternalInput").ap()
    woT = nc.dram_tensor("woT", [M, D], BF16, kind="ExternalInput").ap()
    msk = nc.dram_tensor("msk", [128, 4 * IC], BF16, kind="ExternalInput").ap()
    out = nc.dram_tensor("out", [S, D], BF16, kind="ExternalOutput").ap()

    with tile.TileContext(nc) as tc:
        with (
            tc.tile_pool(name="wts", bufs=1) as wpool,
            tc.tile_pool(name="xs", bufs=1) as xpool,
            tc.tile_pool(name="acts", bufs=1) as apool,
            tc.tile_pool(name="ring", bufs=2) as rpool,
            tc.tile_pool(name="mm", bufs=2, space="PSUM") as mmps,
            tc.tile_pool(name="sc", bufs=2, space="PSUM") as scps,
            tc.tile_pool(name="pv", bufs=2, space="PSUM") as pvps,
        ):
            # --- constants (DMA'd once; outside any bench loop) ---
            wq_sb = wpool.tile([128, 8 * M], XDT, tag="wq")
            wk_sb = wpool.tile([128, 8 * M], XDT, tag="wk")
            wv_sb = wpool.tile([128, 8 * M], XDT, tag="wv")
            wo_sb = wpool.tile([128, 2 * D], PDT, tag="wo")
            msk_sb = wpool.tile([128, 4 * IC], BF16, tag="msk")
            idn = wpool.tile([128, 128], PDT, tag="idn")
            nc.sync.dma_start(wq_sb.rearrange("p (c m) -> p c m", m=M),
                              wqT.rearrange("(c p) m -> p c m", p=128))
            nc.sync.dma_start(wk_sb.rearrange("p (c m) -> p c m", m=M),
                              wkT.rearrange("(c p) m -> p c m", p=128))
            nc.sync.dma_start(wv_sb.rearrange("p (c m) -> p c m", m=M),
                              wvT.rearrange("(c p) m -> p c m", p=128))
            nc.sync.dma_start(wo_sb.rearrange("p (c d) -> p c d", d=D),
                              woT.rearrange("(c p) d -> p c d", p=128))
            nc.sync.dma_start(msk_sb[:], msk[:, :])
            make_identity(nc, idn[:])

            # persistent activation tiles (written each iteration)
            xq_t = xpool.tile([128, 8, S], XDT, tag="xq")
            xk_t = xpool.tile([128, 8, S], XDT, tag="xk")
            kT = apool.tile([128, 2, S], PDT, tag="kT")
            # vc: per (j-tile, head) slot [128, 128]: cols 0-63 v (bf16),
            # cols 64-127 ones (fused softmax row-sum)
            vc = apool.tile([128, NJT, HPC, 128], PDT, tag="vc")
            # ones written once, here (v-halves are overwritten per iter)
            nc.gpsimd.memset(vc[:, :, :, ds(64, 64)], 1.0)

            from contextlib import nullcontext
            loop_cm = (tc.For_i(0, bench_iters, 1)
                       if bench_iters else nullcontext())
            with loop_cm:
                _emit_body(nc, tc, locals())
    nc.finalize()
    return nc


def _qkv_chunk(nc, env, icb):
    """QKV projections for i-chunk icb -> q ring tile, kT cols, vc slots."""
    (xq_t, xk_t, kT, vc, idn) = (env["xq_t"], env["xk_t"], env["kT"],
                                 env["vc"], env["idn"])
    (wq_sb, wk_sb, wv_sb) = (env["wq_sb"], env["wk_sb"], env["wv_sb"])
    mmps, rpool = env["mmps"], env["rpool"]

    q_t = rpool.tile([128, 2, IC], PDT, tag="q", name=f"q_{icb}")
    for which, w_sb, x_t in (("q", wq_sb, xq_t), ("k", wk_sb, xk_t),
                             ("v", wv_sb, xk_t)):
        for mh in range(2):
            ps = mmps.tile([128, IC], F32, tag="mm",
                           name=f"ps_{icb}_{which}_{mh}")
            for dc in range(8):
                nc.tensor.matmul(ps[:],
                                 w_sb.rearrange("p (c m) -> p c m", m=M)[
                                     :, dc, ds(mh * 128, 128)],
                                 x_t[:, dc, ds(icb * IC, IC)],
                                 start=(dc == 0), stop=(dc == 7))
            if which == "q":
                nc.vector.tensor_copy(q_t[:, mh, :], ps[:])
            elif which == "k":
                nc.vector.tensor_copy(kT[:, mh, ds(icb * IC, IC)], ps[:])
            else:
                # v -> [j, m] layout via PE transpose, then into vc slots
                vt_t = rpool.tile([128, IC], PDT, tag="vt",
                                  name=f"vt_{icb}_{mh}")
                nc.vector.tensor_copy(vt_t[:], ps[:])
                tp = mmps.tile([128, IC], PDT, tag="mm",
                               name=f"tp_{icb}_{mh}")
                for t in range(4):
                    nc.tensor.transpose(tp[:, ds(t * 128, 128)],
                                        vt_t[:, ds(t * 128, 128)], idn[:])
                for t in range(4):
                    jt = icb * 4 + t
                    for hh in range(2):
                        h = mh * 2 + hh
                        nc.vector.tensor_copy(
                            vc[:, jt, h, ds(0, 64)],
                            tp[:, ds(t * 128 + hh * 64, 64)])
    return q_t


def _emit_body(nc, tc, env):
    (kT, vc, msk_sb, out, wo_sb) = (
        env["kT"], env["vc"], env["msk_sb"], env["out"], env["wo_sb"])
    (xqT, xkT, xq_t, xk_t) = (env["xqT"], env["xkT"], env["xq_t"],
                              env["xk_t"])
    mmps, scps, pvps, rpool = (env["mmps"], env["scps"], env["pvps"],
                               env["rpool"])

    # x for this iteration: 8 chunk DMAs per tensor, 4KB lines
    for dc in range(8):
        nc.sync.dma_start(xq_t[:, dc, :], xqT[ds(dc * 128, 128), :])
    for dc in range(8):
        nc.sync.dma_start(xk_t[:, dc, :], xkT[ds(dc * 128, 128), :])

    q_next = _qkv_chunk(nc, env, 0)
    for icb in range(NICB):
        q_t = q_next
        # ---- causal attention for this i-chunk: 2 head-pair passes ----
        njt = 4 * icb + 4
        aT = rpool.tile([128, 2, IC], PDT, tag="aT", name=f"aT_{icb}")
        for mh in range(2):  # head pair (2*mh, 2*mh+1)
            pv_t = pvps.tile([128, IC], F32, tag="pv",
                             name=f"pv_{icb}_{mh}")
            pv_o = pvps.tile([128, IC], F32, tag="pv",
                             name=f"pvo_{icb}_{mh}")
            pv_pair = (pv_t, pv_o)
            for jt in range(njt):
                # causal rectangle: diagonal j-tile rr only touches
                # i-cols >= 128*rr (cols < 128*rr never see this jt)
                rr = jt - 4 * icb
                i0 = max(rr, 0) * 128
                w = IC - i0
                s_ps = scps.tile([128, 2 * IC], F32, tag="sc",
                                 name=f"s_{icb}_{mh}_{jt}")
                sv = s_ps.rearrange("p (a b) -> p a b", b=IC)
                for hh in range(2):
                    nc.tensor.matmul(
                        sv[:, hh, ds(i0, w)],
                        kT[ds(hh * 64, 64), mh, ds(jt * 128, 128)],
                        q_t[ds(hh * 64, 64), mh, ds(i0, w)],
                        start=True, stop=True)
                p_t = rpool.tile([128, 2, IC], PDT, tag="p", bufs=3,
                                 name=f"p_{icb}_{mh}_{jt}")
                nc.scalar.activation(
                    p_t[:, :, ds(i0, w)], sv[:, :, ds(i0, w)], AF.Exp)
                if rr >= 0:  # diagonal block: causal mask
                    for hh in range(2):
                        nc.vector.tensor_tensor(
                            p_t[:, hh, ds(i0, w)], p_t[:, hh, ds(i0, w)],
                            msk_sb[:, ds(rr * IC + i0, w)], ALU.mult)
                for hh in range(2):
                    nc.tensor.matmul(
                        pv_pair[hh][:, ds(i0, w)],
                        vc[:, jt, mh * 2 + hh, :],
                        p_t[:, hh, ds(i0, w)],
                        start=(jt == 0), stop=(jt == njt - 1))
            for hh in range(2):
                rc_t = rpool.tile([64, IC], F32, tag="rc",
                                  name=f"rc_{icb}_{mh}_{hh}")
                nc.vector.reciprocal(rc_t[:], pv_pair[hh][ds(64, 64), :])
                nc.vector.tensor_tensor(
                    aT[ds(hh * 64, 64), mh, :],
                    pv_pair[hh][ds(0, 64), :],
                    rc_t[:], ALU.mult)

        # ---- QKV for the next i-chunk (overlaps ACT-paced attention) ----
        if icb + 1 < NICB:
            q_next = _qkv_chunk(nc, env, icb + 1)

        # ---- partial out-projection for this i-chunk ----
        for i2 in range(2):
            o_sb = rpool.tile([128, 2, D], PDT, tag="osb",
                              name=f"osb_{icb}_{i2}")
            for u in range(2):
                i128 = i2 * 2 + u
                for dn in range(2):
                    o_ps = mmps.tile([128, IC], F32, tag="mm",
                                     name=f"op_{icb}_{i128}_{dn}")
                    for mc in range(2):
                        nc.tensor.matmul(
                            o_ps[:],
                            aT[:, mc, ds(i128 * 128, 128)],
                            wo_sb.rearrange("p (c d) -> p c d", d=D)[
                                :, mc, ds(dn * IC, IC)],
                            start=(mc == 0), stop=(mc == 1))
                    if dn == 0:
                        nc.vector.tensor_copy(o_sb[:, u, ds(0, IC)], o_ps[:])
                    else:
                        nc.scalar.copy(o_sb[:, u, ds(IC, IC)], o_ps[:])
            nc.sync.dma_start(
                out[ds(icb * IC + i2 * 256, 256), :].rearrange(
                    "(a p) d -> p a d", p=128),
                o_sb[:])


_STATE = {}


def _get_runner(bench_iters=None):
    """Build the Bass module and a cached jitted SPMD executor (compile once)."""
    global _STATE
    if bench_iters in _STATE:
        return _STATE[bench_iters]

    import jax
    from jax.sharding import Mesh, PartitionSpec
    from jax.experimental.shard_map import shard_map
    from concourse import bass2jax

    bass2jax.install_neuronx_cc_hook()
    nc = _build_bass(bench_iters)

    partition_name = (nc.partition_id_tensor.name
                      if nc.partition_id_tensor else None)
    in_names, out_names, out_avals, zero_shapes = [], [], [], []
    for alloc in nc.m.functions[0].allocations:
        if not isinstance(alloc, mybir.MemoryLocationSet):
            continue
        name = alloc.memorylocations[0].name
        if alloc.kind == "ExternalInput":
            if name != partition_name:
                in_names.append(name)
        elif alloc.kind == "ExternalOutput":
            shape = tuple(alloc.tensor_shape)
            dtype = mybir.dt.np(alloc.dtype)
            out_names.append(name)
            out_avals.append(jax.core.ShapedArray(shape, dtype))
            zero_shapes.append((shape, dtype))
    n_params = len(in_names)
    n_outs = len(out_avals)
    all_in_names = list(in_names) + list(out_names)
    if partition_name is not None:
        all_in_names.append(partition_name)

    def _body(*args):
        operands = list(args)
        if partition_name is not None:
            operands.append(bass2jax.partition_id_tensor())
        outs = bass2jax._bass_exec_p.bind(
            *operands,
            out_avals=tuple(out_avals),
            in_names=tuple(all_in_names),
            out_names=tuple(out_names),
            lowering_input_output_aliases=(),
            sim_require_finite=True,
            sim_require_nnan=True,
            nc=nc,
        )
        return tuple(outs)

    devices = jax.devices()[:NCORES]
    mesh = Mesh(np.asarray(devices), ("core",))
    in_specs = (PartitionSpec("core"),) * (n_params + n_outs)
    out_specs = (PartitionSpec("core"),) * n_outs
    donate = tuple(range(n_params, n_params + n_outs))
    sharded = jax.jit(
        shard_map(_body, mesh=mesh, in_specs=in_specs, out_specs=out_specs,
                  check_rep=False),
        donate_argnums=donate, keep_unused=True)

    def run(in_maps):
        concat_in = [
            np.concatenate([np.asarray(in_maps[c][k]) for c in range(NCORES)],
                           axis=0)
            for k in in_names
        ]
        concat_zeros = [np.zeros((NCORES * s[0], *s[1:]), dt)
                        for s, dt in zero_shapes]
        out_arrs = sharded(*concat_in, *concat_zeros)
        return [
            {k: np.asarray(out_arrs[i]).reshape(NCORES, *out_avals[i].shape)[c]
             for i, k in enumerate(out_names)}
            for c in range(NCORES)
        ]

    _STATE[bench_iters] = run
    return run


def _make_mask():
    """msk[jj, rr*512 + ii] = 1 if ii >= jj + 128*rr else 0 (multiplicative)."""
    import ml_dtypes
    jj = np.arange(128)[:, None]
    ii = np.arange(IC)[None, :]
    tiles = [np.where(ii >= jj + 128 * rr, 1.0, 0.0)
             for rr in range(4)]
    return np.concatenate(tiles, axis=1).astype(ml_dtypes.bfloat16)


def prepare_in_maps(inputs_q, inputs_kv, Wq, Wk, Wv, Wo):
    import ml_dtypes
    xdt = ml_dtypes.bfloat16
    Wq = np.asarray(Wq, np.float32)
    Wk = np.asarray(Wk, np.float32)
    Wv = np.asarray(Wv, np.float32)
    Wo = np.asarray(Wo, np.float32)
    msk = _make_mask()
    scale = 1.0 / np.sqrt(np.float32(Dh))
    xq_b = [np.ascontiguousarray(
        np.asarray(inputs_q, np.float32)[b].T.astype(xdt)) for b in range(B)]
    xk_b = [np.ascontiguousarray(
        np.asarray(inputs_kv, np.float32)[b].T.astype(xdt)) for b in range(B)]
    in_maps = []
    for c in range(NCORES):
        b, g = c // 4, c % 4
        sl = slice(g * M, (g + 1) * M)
        in_maps.append({
            "xqT": xq_b[b],
            "xkT": xk_b[b],
            "wqT": np.ascontiguousarray((Wq[sl, :] * scale).T.astype(xdt)),
            "wkT": np.ascontiguousarray(Wk[sl, :].T.astype(xdt)),
            "wvT": np.ascontiguousarray(Wv[sl, :].T.astype(xdt)),
            "woT": np.ascontiguousarray(Wo[:, sl].T.astype(xdt)),
            "msk": msk,
        })
    return in_maps


def _run_fallback(in_maps):
    """Slow-but-sure path: the stock SPMD runner (fresh compile per call)."""
    from concourse.bass_utils import run_bass_kernel_spmd
    nc = _build_bass()
    res = run_bass_kernel_spmd(nc, in_maps, core_ids=list(range(NCORES)))
    return res.results


def kernel(inputs_q, inputs_kv, mask, Wq, Wk, Wv, Wo):
    in_maps = prepare_in_maps(inputs_q, inputs_kv, Wq, Wk, Wv, Wo)
    try:
        results = _get_runner()(in_maps)
    except Exception:
        results = _run_fallback(in_maps)
    full = np.empty((B, S, D), np.float32)
    for b in range(B):
        acc = results[4 * b]["out"].astype(np.float32)
        for g in range(1, 4):
            acc = acc + results[4 * b + g]["out"]
        full[b] = acc
    return full


# revision 14
# speedup vs baseline: 1.4425x; 1.4425x over previous
"""Multi-head causal attention (B=2, S=2048, H=16, Dh=64) on 8 TRN2 NeuronCores.

Sharding: (batch, head-quad) — core c owns batch c//4 and heads
4*(c%4)..4*(c%4)+3 (a 256-wide feature block). Each core computes its
heads' QKV projections over its batch's 2048 tokens, causal attention,
and a partial output projection (attn_c @ Wo[:, mslice].T); the host
sums the 4 partials per batch. vs the previous 2-head x 2-batch TP
sharding this halves both the x reads (8MB not 16MB) and the fp32
partial-out writes (8MB not 16MB) per core -- DMA was the measured
bottleneck (~150GB/s effective on 1-2KB-line transfers).

On-chip: feature dim on SBUF partitions throughout. Scores are computed
as S.T[j, i] per 128-j-tile x 512-i-chunk; the two heads of a pair live
on partitions 0-63 / 64-127 so their score matmuls go to disjoint PE
row-groups and run concurrently. exp is one merged [128,1024] ACT
instruction per (jt, head-pair) (amortizes the 352-cycle ACT fixed
cost). p and v are bf16 (full-rate PE, half-cost DVE mask); softmax
row-sum comes free from 64 ones-columns appended to V in the PV matmul.
"""

import os

import numpy as np

import concourse.bass as bass
import concourse.mybir as mybir
import concourse.tile as tile
from concourse import bacc
from concourse.bass import ds
from concourse.masks import make_identity

B, S, H, Dh = 2, 2048, 16, 64
D = H * Dh            # 1024
NCORES = 8
HPC = 4               # heads per core
M = HPC * Dh          # per-core feature block = 256
IC = 512              # i-chunk (matmul moving free dim)
NICB = S // IC        # 4 i-chunks
NJT = S // 128        # 16 j-tiles

F32 = mybir.dt.float32
F32R = mybir.dt.float32r
BF16 = mybir.dt.bfloat16

XDT = BF16            # x activations + QKV weights (DMA + full-rate PE)
PDT = BF16            # exp(p) and v in the PV matmul
ODT = F32             # output partials must stay f32 (2-byte DMA writes slow)
AF = mybir.ActivationFunctionType
ALU = mybir.AluOpType
ABLATE = os.environ.get("ABLATE", "")


def _build_bass(bench_iters=None):
    nc = bacc.Bacc("TRN2", target_bir_lowering=False, debug=False,
                   num_devices=NCORES)

    xqT = nc.dram_tensor("xqT", [D, S], XDT, kind="ExternalInput").ap()
    xkT = nc.dram_tensor("xkT", [D, S], XDT, kind="ExternalInput").ap()
    wqT = nc.dram_tensor("wqT", [D, M], XDT, kind="ExternalInput").ap()
    wkT = nc.dram_tensor("wkT", [D, M], XDT, kind="ExternalInput").ap()
    wvT = nc.dram_tensor("wvT", [D, M], XDT, kind="ExternalInput").ap()
    woT = nc.dram_tensor("woT", [M, D], BF16, kind="ExternalInput").ap()
    msk = nc.dram_tensor("msk", [128, 4 * IC], BF16, kind="ExternalInput").ap()
    out = nc.dram_tensor("out", [S, D], F32, kind="ExternalOutput").ap()

    with tile.TileContext(nc) as tc:
        with (
            tc.tile_pool(name="wts", bufs=1) as wpool,
            tc.tile_pool(name="xs", bufs=1) as xpool,
            tc.tile_pool(name="acts", bufs=1) as apool,
            tc.tile_pool(name="ring", bufs=2) as rpool,
            tc.tile_pool(name="mm", bufs=2, space="PSUM") as mmps,
            tc.tile_pool(name="sc", bufs=2, space="PSUM") as scps,
            tc.tile_pool(name="pv", bufs=2, space="PSUM") as pvps,
        ):
            # --- constants (DMA'd once; outside any bench loop) ---
            wq_sb = wpool.tile([128, 8 * M], XDT, tag="wq")
            wk_sb = wpool.tile([128, 8 * M], XDT, tag="wk")
            wv_sb = wpool.tile([128, 8 * M], XDT, tag="wv")
            wo_sb = wpool.tile([128, 2 * D], PDT, tag="wo")
            msk_sb = wpool.tile([128, 4 * IC], BF16, tag="msk")
            idn = wpool.tile([128, 128], PDT, tag="idn")
            nc.sync.dma_start(wq_sb.rearrange("p (c m) -> p c m", m=M),
                              wqT.rearrange("(c p) m -> p c m", p=128))
            nc.sync.dma_start(wk_sb.rearrange("p (c m) -> p c m", m=M),
                              wkT.rearrange("(c p) m -> p c m", p=128))
            nc.sync.dma_start(wv_sb.rearrange("p (c m) -> p c m", m=M),
                              wvT.rearrange("(c p) m -> p c m", p=128))
            nc.sync.dma_start(wo_sb.rearrange("p (c d) -> p c d", d=D),
                              woT.rearrange("(c p) d -> p c d", p=128))
            nc.sync.dma_start(msk_sb[:], msk[:, :])
            make_identity(nc, idn[:])

            # persistent activation tiles (written each iteration)
            xq_t = xpool.tile([128, 8, S], XDT, tag="xq")
            xk_t = xpool.tile([128, 8, S], XDT, tag="xk")
            kT = apool.tile([128, 2, S], PDT, tag="kT")
            # vc: per (j-tile, head) slot [128, 128]: cols 0-63 v (bf16),
            # cols 64-127 ones (fused softmax row-sum)
            vc = apool.tile([128, NJT, HPC, 128], PDT, tag="vc")
            # ones written once, here (v-halves are overwritten per iter)
            nc.gpsimd.memset(vc[:, :, :, ds(64, 64)], 1.0)

            from contextlib import nullcontext
            loop_cm = (tc.For_i(0, bench_iters, 1)
                       if bench_iters else nullcontext())
            with loop_cm:
                _emit_body(nc, tc, locals())
    nc.finalize()
    return nc


def _qkv_chunk(nc, env, icb):
    """QKV projections for i-chunk icb -> q ring tile, kT cols, vc slots."""
    (xq_t, xk_t, kT, vc, idn) = (env["xq_t"], env["xk_t"], env["kT"],
                                 env["vc"], env["idn"])
    (wq_sb, wk_sb, wv_sb) = (env["wq_sb"], env["wk_sb"], env["wv_sb"])
    mmps, rpool = env["mmps"], env["rpool"]

    q_t = rpool.tile([128, 2, IC], PDT, tag="q", bufs=4,
                     name=f"q_{icb}")
    for which, w_sb, x_t in (("q", wq_sb, xq_t), ("k", wk_sb, xk_t),
                             ("v", wv_sb, xk_t)):
        for mh in range(2):
            ps = mmps.tile([128, IC], F32, tag="mm",
                           name=f"ps_{icb}_{which}_{mh}")
            ndc = 1 if ABLATE == "qkv1" else 8
            for dc in range(ndc):
                nc.tensor.matmul(ps[:],
                                 w_sb.rearrange("p (c m) -> p c m", m=M)[
                                     :, dc, ds(mh * 128, 128)],
                                 x_t[:, dc, ds(icb * IC, IC)],
                                 start=(dc == 0), stop=(dc == ndc - 1))
            if which == "q":
                nc.vector.tensor_copy(q_t[:, mh, :], ps[:])
            elif which == "k":
                nc.vector.tensor_copy(kT[:, mh, ds(icb * IC, IC)], ps[:])
            else:
                # v -> [j, m] layout via PE transpose, then into vc slots
                vt_t = rpool.tile([128, IC], PDT, tag="vt",
                                  name=f"vt_{icb}_{mh}")
                nc.vector.tensor_copy(vt_t[:], ps[:])
                tp = mmps.tile([128, IC], PDT, tag="mm",
                               name=f"tp_{icb}_{mh}")
                for t in range(4):
                    nc.tensor.transpose(tp[:, ds(t * 128, 128)],
                                        vt_t[:, ds(t * 128, 128)], idn[:])
                for t in range(4):
                    jt = icb * 4 + t
                    for hh in range(2):
                        h = mh * 2 + hh
                        nc.vector.tensor_copy(
                            vc[:, jt, h, ds(0, 64)],
                            tp[:, ds(t * 128 + hh * 64, 64)])
    return q_t


def _emit_body(nc, tc, env):
    (kT, vc, msk_sb, out, wo_sb) = (
        env["kT"], env["vc"], env["msk_sb"], env["out"], env["wo_sb"])
    (xqT, xkT, xq_t, xk_t) = (env["xqT"], env["xkT"], env["xq_t"],
                              env["xk_t"])
    mmps, scps, pvps, rpool = (env["mmps"], env["scps"], env["pvps"],
                               env["rpool"])

    # x for this iteration: 8 chunk DMAs per tensor, 4KB lines
    for dc in range(8):
        nc.sync.dma_start(xq_t[:, dc, :], xqT[ds(dc * 128, 128), :])
    if ABLATE != "noxk":
        for dc in range(8):
            nc.sync.dma_start(xk_t[:, dc, :], xkT[ds(dc * 128, 128), :])

    q_tiles = [_qkv_chunk(nc, env, icb) for icb in range(NICB)]
    for icb in range(NICB):
        q_t = q_tiles[icb]
        # ---- causal attention for this i-chunk: 2 head-pair passes ----
        njt = 4 * icb + 4
        aT = rpool.tile([128, 2, IC], PDT, tag="aT", name=f"aT_{icb}")
        for mh in range(2):  # head pair (2*mh, 2*mh+1)
            pv_t = pvps.tile([128, IC], F32, tag="pv",
                             name=f"pv_{icb}_{mh}")
            pv_o = pvps.tile([128, IC], F32, tag="pv",
                             name=f"pvo_{icb}_{mh}")
            pv_pair = (pv_t, pv_o)
            for jt in range(njt):
                # causal rectangle: diagonal j-tile rr only touches
                # i-cols >= 128*rr (cols < 128*rr never see this jt)
                rr = jt - 4 * icb
                i0 = max(rr, 0) * 128
                w = IC - i0
                s_ps = scps.tile([128, 2 * IC], F32, tag="sc",
                                 name=f"s_{icb}_{mh}_{jt}")
                sv = s_ps.rearrange("p (a b) -> p a b", b=IC)
                sw = 64 if ABLATE == "sc64" else w
                for hh in range(2):
                    nc.tensor.matmul(
                        sv[:, hh, ds(i0, sw)],
                        kT[ds(hh * 64, 64), mh, ds(jt * 128, 128)],
                        q_t[ds(hh * 64, 64), mh, ds(i0, sw)],
                        start=True, stop=True)
                p_t = rpool.tile([128, 2, IC], PDT, tag="p", bufs=3,
                                 name=f"p_{icb}_{mh}_{jt}")
                if ABLATE == "act64":
                    nc.scalar.activation(
                        p_t[:, :, ds(i0, 64)], sv[:, :, ds(i0, 64)], AF.Exp)
                else:
                    nc.scalar.activation(
                        p_t[:, :, ds(i0, w)], sv[:, :, ds(i0, w)], AF.Exp)
                if rr >= 0:  # diagonal block: causal mask
                    for hh in range(2):
                        nc.vector.tensor_tensor(
                            p_t[:, hh, ds(i0, w)], p_t[:, hh, ds(i0, w)],
                            msk_sb[:, ds(rr * IC + i0, w)], ALU.mult)
                pw = 64 if ABLATE == "pv64" else w
                for hh in range(2):
                    nc.tensor.matmul(
                        pv_pair[hh][:, ds(i0, pw)] if pw != w
                        else pv_pair[hh][:, ds(i0, w)],
                        vc[:, jt, mh * 2 + hh, :],
                        p_t[:, hh, ds(i0, pw)],
                        start=(jt == 0), stop=(jt == njt - 1))
            for hh in range(2):
                rc_t = rpool.tile([64, IC], F32, tag="rc",
                                  name=f"rc_{icb}_{mh}_{hh}")
                nc.vector.reciprocal(rc_t[:], pv_pair[hh][ds(64, 64), :])
                nc.vector.tensor_tensor(
                    aT[ds(hh * 64, 64), mh, :],
                    pv_pair[hh][ds(0, 64), :],
                    rc_t[:], ALU.mult)

        # ---- partial out-projection for this i-chunk ----
        for i2 in range(2):
            o_sb = rpool.tile([128, 2, D], F32, tag="osb",
                              name=f"osb_{icb}_{i2}")
            for u in range(2):
                i128 = i2 * 2 + u
                for dn in range(2):
                    o_ps = mmps.tile([128, IC], F32, tag="mm",
                                     name=f"op_{icb}_{i128}_{dn}")
                    for mc in range(2):
                        nc.tensor.matmul(
                            o_ps[:],
                            aT[:, mc, ds(i128 * 128, 128)],
                            wo_sb.rearrange("p (c d) -> p c d", d=D)[
                                :, mc, ds(dn * IC, IC)],
                            start=(mc == 0), stop=(mc == 1))
                    nc.vector.tensor_copy(o_sb[:, u, ds(dn * IC, IC)],
                                          o_ps[:])
            if ABLATE != "noout":
                nc.sync.dma_start(
                    out[ds(icb * IC + i2 * 256, 256), :].rearrange(
                        "(a p) d -> p a d", p=128),
                    o_sb[:])


_STATE = {}


def _get_runner(bench_iters=None):
    """Build the Bass module and a cached jitted SPMD executor (compile once)."""
    global _STATE
    if bench_iters in _STATE:
        return _STATE[bench_iters]

    import jax
    from jax.sharding import Mesh, PartitionSpec
    from jax.experimental.shard_map import shard_map
    from concourse import bass2jax

    bass2jax.install_neuronx_cc_hook()
    nc = _build_bass(bench_iters)

    partition_name = (nc.partition_id_tensor.name
                      if nc.partition_id_tensor else None)
    in_names, out_names, out_avals, zero_shapes = [], [], [], []
    for alloc in nc.m.functions[0].allocations:
        if not isinstance(alloc, mybir.MemoryLocationSet):
            continue
        name = alloc.memorylocations[0].name
        if alloc.kind == "ExternalInput":
            if name != partition_name:
                in_names.append(name)
        elif alloc.kind == "ExternalOutput":
            shape = tuple(alloc.tensor_shape)
            dtype = mybir.dt.np(alloc.dtype)
            out_names.append(name)
            out_avals.append(jax.core.ShapedArray(shape, dtype))
            zero_shapes.append((shape, dtype))
    n_params = len(in_names)
    n_outs = len(out_avals)
    all_in_names = list(in_names) + list(out_names)
    if partition_name is not None:
        all_in_names.append(partition_name)

    def _body(*args):
        operands = list(args)
        if partition_name is not None:
            operands.append(bass2jax.partition_id_tensor())
        outs = bass2jax._bass_exec_p.bind(
            *operands,
            out_avals=tuple(out_avals),
            in_names=tuple(all_in_names),
            out_names=tuple(out_names),
            lowering_input_output_aliases=(),
            sim_require_finite=True,
            sim_require_nnan=True,
            nc=nc,
        )
        return tuple(outs)

    devices = jax.devices()[:NCORES]
    mesh = Mesh(np.asarray(devices), ("core",))
    in_specs = (PartitionSpec("core"),) * (n_params + n_outs)
    out_specs = (PartitionSpec("core"),) * n_outs
    donate = tuple(range(n_params, n_params + n_outs))
    sharded = jax.jit(
        shard_map(_body, mesh=mesh, in_specs=in_specs, out_specs=out_specs,
                  check_rep=False),
        donate_argnums=donate, keep_unused=True)

    def run(in_maps):
        concat_in = [
            np.concatenate([np.asarray(in_maps[c][k]) for c in range(NCORES)],
                           axis=0)
            for k in in_names
        ]
        concat_zeros = [np.zeros((NCORES * s[0], *s[1:]), dt)
                        for s, dt in zero_shapes]
        out_arrs = sharded(*concat_in, *concat_zeros)
        return [
            {k: np.asarray(out_arrs[i]).reshape(NCORES, *out_avals[i].shape)[c]
             for i, k in enumerate(out_names)}
            for c in range(NCORES)
        ]

    _STATE[bench_iters] = run
    return run


def _make_mask():
    """msk[jj, rr*512 + ii] = 1 if ii >= jj + 128*rr else 0 (multiplicative)."""
    import ml_dtypes
    jj = np.arange(128)[:, None]
    ii = np.arange(IC)[None, :]
    tiles = [np.where(ii >= jj + 128 * rr, 1.0, 0.0)
             for rr in range(4)]
    return np.concatenate(tiles, axis=1).astype(ml_dtypes.bfloat16)


def prepare_in_maps(inputs_q, inputs_kv, Wq, Wk, Wv, Wo):
    import ml_dtypes
    xdt = ml_dtypes.bfloat16
    Wq = np.asarray(Wq, np.float32)
    Wk = np.asarray(Wk, np.float32)
    Wv = np.asarray(Wv, np.float32)
    Wo = np.asarray(Wo, np.float32)
    msk = _make_mask()
    scale = 1.0 / np.sqrt(np.float32(Dh))
    xq_b = [np.ascontiguousarray(
        np.asarray(inputs_q, np.float32)[b].T.astype(xdt)) for b in range(B)]
    xk_b = [np.ascontiguousarray(
        np.asarray(inputs_kv, np.float32)[b].T.astype(xdt)) for b in range(B)]
    in_maps = []
    for c in range(NCORES):
        b, g = c // 4, c % 4
        sl = slice(g * M, (g + 1) * M)
        in_maps.append({
            "xqT": xq_b[b],
            "xkT": xk_b[b],
            "wqT": np.ascontiguousarray((Wq[sl, :] * scale).T.astype(xdt)),
            "wkT": np.ascontiguousarray(Wk[sl, :].T.astype(xdt)),
            "wvT": np.ascontiguousarray(Wv[sl, :].T.astype(xdt)),
            "woT": np.ascontiguousarray(Wo[:, sl].T.astype(xdt)),
            "msk": msk,
        })
    return in_maps


def _run_fallback(in_maps):
    """Slow-but-sure path: the stock SPMD runner (fresh compile per call)."""
    from concourse.bass_utils import run_bass_kernel_spmd
    nc = _build_bass()
    res = run_bass_kernel_spmd(nc, in_maps, core_ids=list(range(NCORES)))
    return res.results


def kernel(inputs_q, inputs_kv, mask, Wq, Wk, Wv, Wo):
    in_maps = prepare_in_maps(inputs_q, inputs_kv, Wq, Wk, Wv, Wo)
    try:
        results = _get_runner()(in_maps)
    except Exception:
        results = _run_fallback(in_maps)
    full = np.empty((B, S, D), np.float32)
    for b in range(B):
        acc = results[4 * b]["out"].astype(np.float32)
        for g in range(1, 4):
            acc = acc + results[4 * b + g]["out"]
        full[b] = acc
    return full
